# revision 10
# baseline (speedup 1.0000x reference)
# kernel.py — "Show, Attend and Tell" decoder on 8 trn2 NeuronCores.
# Batch-sharded SPMD: core c owns sorted sequences c, c+8, ..., c+120 (16 each);
# no cross-core traffic. Host does index/sort/gather glue; device does the heavy
# compute: att1 projection, 31-step attention-LSTM scan, vocab FC.
#
# Key device-side tricks:
#  * full_att_w folded into enc/dec attention weights (|w| into the projections,
#    sign into the A-reduction matmul via a host-built block-diagonal constant),
#    so score = sum_a sign_a * max((att1w+att2w)_a, 0) needs one fused
#    tensor_scalar(add,max) per (seq, a-tile) and a PE reduction that lands
#    batch-major in PSUM.
#  * softmax normalization deferred: PE contracts unnormalized exp(score-max)
#    against image values; 1/Z applied as a per-partition scalar afterwards.
#  * alpha placed into a zero-padded DRAM "block diagonal" with one plain DMA,
#    then transpose-loaded so ctx for all active seqs is a single dense
#    accumulation over packed (seq, p-half) K-tiles.
#  * LSTM biases folded in via an ones-row appended to h^T (K=1 matmul rows).
#  * h kept fp32 + bf16 mirror; gates/y transposed b-major<->d-major via
#    DRAM-bounce transpose DMAs (xbar).
import math
from contextlib import ExitStack

import numpy as np
import ml_dtypes

B, L, GRID, D, P = 128, 32, 14, 2048, 196
A, E, H, V = 512, 512, 512, 10000
T = L - 1
NCORES = 8
BPC = B // NCORES
PP = 256
BF16 = ml_dtypes.bfloat16
KD, KA, KH = D // 128, A // 128, H // 128
NG = 4 * H
VT = (V + 127) // 128

_CACHE = {}


def _host_prep(inputs):
    lengths = np.asarray(inputs["lengths"])
    captions = np.asarray(inputs["captions"])
    img = np.asarray(inputs["image_embeddings"], dtype=np.float32).reshape(B, P, D)

    order = np.argsort(-lengths, kind="stable").astype(np.int32)
    lengths_s = np.asarray(lengths)[order]
    caps = captions[order]
    dec_len = (lengths_s - 1).astype(np.int32)
    img_s = img[order]

    emb = np.asarray(inputs["emb_table"], dtype=np.float32)[caps]
    mean_img = img_s.mean(axis=1)
    h0 = mean_img @ np.asarray(inputs["init_h_w"], np.float32) + np.asarray(inputs["init_h_b"], np.float32)
    c0 = mean_img @ np.asarray(inputs["init_c_w"], np.float32) + np.asarray(inputs["init_c_b"], np.float32)

    wcol = np.asarray(inputs["full_att_w"], np.float32)[:, 0]
    wabs = np.abs(wcol)
    signs = np.sign(wcol).astype(np.float32)
    enc_att_wf = np.asarray(inputs["enc_att_w"], np.float32) * wabs[None, :]
    att1_bias = (np.asarray(inputs["enc_att_b"], np.float32)
                 + np.asarray(inputs["dec_att_b"], np.float32)) * wabs
    dec_att_wf = np.asarray(inputs["dec_att_w"], np.float32) * wabs[None, :]

    signs_diag = np.zeros((128, KA, BPC, BPC), np.float32)
    for b in range(BPC):
        signs_diag[:, :, b, b] = signs.reshape(KA, 128).T

    w_ih = np.asarray(inputs["w_ih"], np.float32)
    w_ih_e = w_ih[:E, :]
    W2 = w_ih[E:, :]
    b_gates = np.asarray(inputs["b_ih"], np.float32) + np.asarray(inputs["b_hh"], np.float32)
    f_beta_aug = np.zeros((H + 128, D), np.float32)
    f_beta_aug[:H] = np.asarray(inputs["f_beta_w"], np.float32)
    f_beta_aug[H] = np.asarray(inputs["f_beta_b"], np.float32)

    fc_wp = np.zeros((H, VT * 128), np.float32)
    fc_wp[:, :V] = np.asarray(inputs["fc_w"], np.float32)
    fc_bp = np.zeros((VT * 128,), np.float32)
    fc_bp[:V] = np.asarray(inputs["fc_b"], np.float32)

    n_t = [int((dec_len > t).sum()) for t in range(T)]
    ns = [max(1, min(BPC, math.ceil(nt / NCORES))) for nt in n_t]
    roff = np.cumsum([0] + ns).astype(np.int64)
    R = int(roff[-1])

    per_core = []
    for c in range(NCORES):
        idx = np.arange(c, B, NCORES)
        img_pad = np.zeros((BPC, PP, D), dtype=BF16)
        img_pad[:, :P, :] = img_s[idx].astype(BF16)
        embT = np.ascontiguousarray(
            emb[idx][:, :T, :].transpose(2, 1, 0).reshape(E, T * BPC)).astype(BF16)
        per_core.append({
            "img": img_pad,
            "embT": embT,
            "h0T": np.ascontiguousarray(h0[idx].T).astype(np.float32),
            "c0T": np.ascontiguousarray(c0[idx].T).astype(np.float32),
        })

    shared = {
        "enc_att_wf": enc_att_wf.astype(BF16),
        "att1_bias": att1_bias.astype(np.float32),
        "dec_att_wf": dec_att_wf.astype(BF16),
        "signs_diag": signs_diag.astype(BF16),
        "w_ih_e": w_ih_e.astype(BF16),
        "W2": W2.astype(BF16),
        "w_hh": np.asarray(inputs["w_hh"], np.float32).astype(BF16),
        "b_gates": b_gates.astype(np.float32),
        "f_beta_aug": f_beta_aug.astype(BF16),
        "fc_w": fc_wp.astype(BF16),
        "fc_b": fc_bp.astype(np.float32),
        "alpha_diag_zero": np.zeros((2 * BPC, BPC, 128), BF16),
    }
    meta = {"order": order, "caps": caps, "dec_len": dec_len,
            "ns": ns, "roff": roff, "R": R}
    return shared, per_core, meta


def _build_program(ns, roff, R):
    import concourse.bass as bass
    import concourse.mybir as mybir
    import concourse.tile as tile
    from concourse import bacc
    from concourse import tile_utils
    tile_utils.max_sbuf_usage = 212 * 1024

    dt = mybir.dt
    AF = mybir.ActivationFunctionType
    OP = mybir.AluOpType
    AX = mybir.AxisListType

    nc = bacc.Bacc("TRN2", target_bir_lowering=False, debug=False,
                   enable_asserts=False, num_devices=NCORES)

    d_img = nc.dram_tensor("img", [BPC, PP, D], dt.bfloat16, kind="ExternalInput")
    d_embT = nc.dram_tensor("embT", [E, T * BPC], dt.bfloat16, kind="ExternalInput")
    d_h0T = nc.dram_tensor("h0T", [H, BPC], dt.float32, kind="ExternalInput")
    d_c0T = nc.dram_tensor("c0T", [H, BPC], dt.float32, kind="ExternalInput")
    d_encw = nc.dram_tensor("enc_att_wf", [D, A], dt.bfloat16, kind="ExternalInput")
    d_att1b = nc.dram_tensor("att1_bias", [A], dt.float32, kind="ExternalInput")
    d_decw = nc.dram_tensor("dec_att_wf", [H, A], dt.bfloat16, kind="ExternalInput")
    d_sdiag = nc.dram_tensor("signs_diag", [128, KA, BPC, BPC], dt.bfloat16, kind="ExternalInput")
    d_wihe = nc.dram_tensor("w_ih_e", [E, NG], dt.bfloat16, kind="ExternalInput")
    d_W2 = nc.dram_tensor("W2", [D, NG], dt.bfloat16, kind="ExternalInput")
    d_whh = nc.dram_tensor("w_hh", [H, NG], dt.bfloat16, kind="ExternalInput")
    d_bg = nc.dram_tensor("b_gates", [NG], dt.float32, kind="ExternalInput")
    d_fbeta = nc.dram_tensor("f_beta_aug", [H + 128, D], dt.bfloat16, kind="ExternalInput")
    d_fcw = nc.dram_tensor("fc_w", [H, VT * 128], dt.bfloat16, kind="ExternalInput")
    d_fcb = nc.dram_tensor("fc_b", [VT * 128], dt.float32, kind="ExternalInput")
    d_adz = nc.dram_tensor("alpha_diag_zero", [2 * BPC, BPC, 128], dt.bfloat16, kind="ExternalInput")

    d_predsT = nc.dram_tensor("predsT", [VT * 128, R], dt.float32, kind="ExternalOutput")
    d_alphas = nc.dram_tensor("alphas", [T, BPC, P], dt.float32, kind="ExternalOutput")

    d_att1w = nc.dram_tensor("att1w_scr", [BPC, KA, 128, P], dt.bfloat16, kind="Internal")
    d_embpre = nc.dram_tensor("embpre_scr", [KD, 128, T * BPC], dt.bfloat16, kind="Internal")
    d_adiag = nc.dram_tensor("alpha_diag", [2 * BPC, BPC, 128], dt.bfloat16, kind="Internal")
    d_ybounce = nc.dram_tensor("y_bounce", [32, NG], dt.bfloat16, kind="Internal")
    d_gbounce = nc.dram_tensor("g_bounce", [32, NG], dt.bfloat16, kind="Internal")

    steps = len(ns)

    with tile.TileContext(nc) as tc, ExitStack() as ctx:
        pers = ctx.enter_context(tc.tile_pool(name="pers", bufs=1))
        sb_hT = pers.tile([128, KH, BPC], dt.float32, tag="hT")
        sb_hTb = pers.tile([128, KH + 1, BPC], dt.bfloat16, tag="hTb")
        sb_cT = pers.tile([128, KH, BPC], dt.float32, tag="cT")
        sb_hist = pers.tile([128, KH, R], dt.bfloat16, tag="hist")
        sb_decw = pers.tile([128, KH, A], dt.bfloat16, tag="decw")
        sb_sdiag = pers.tile([128, KA, BPC, BPC], dt.bfloat16, tag="sdiag")
        sb_whh = pers.tile([128, KH, NG], dt.bfloat16, tag="whh")
        sb_fbe = pers.tile([128, KH + 1, D], dt.bfloat16, tag="fbe")
        sb_Ablk = pers.tile([128, 2 * BPC, BPC], dt.bfloat16, tag="Ablk")

        nc.sync.dma_start(sb_decw[:], d_decw.ap().rearrange("(k p) a -> p k a", p=128))
        nc.sync.dma_start(sb_sdiag[:], d_sdiag.ap())
        nc.sync.dma_start(sb_whh[:], d_whh.ap().rearrange("(k p) g -> p k g", p=128, k=KH))
        nc.sync.dma_start(sb_fbe[:], d_fbeta.ap().rearrange("(k p) g -> p k g", p=128, k=KH + 1))
        nc.sync.dma_start(sb_hT[:], d_h0T.ap().rearrange("(k p) b -> p k b", p=128))
        nc.sync.dma_start(sb_cT[:], d_c0T.ap().rearrange("(k p) b -> p k b", p=128))
        nc.vector.memset(sb_hTb[:, KH, :], 0.0)
        nc.vector.memset(sb_hTb[:1, KH, :], 1.0)
        nc.vector.tensor_copy(sb_hTb[:, :KH, :], sb_hT[:])
        nc.sync.dma_start(d_adiag.ap(), d_adz.ap())
        nc.sync.dma_start(d_ybounce.ap().rearrange("a b -> (a b)"),
                          d_adz.ap().rearrange("a b c -> (a b c)"))
        nc.sync.dma_start(d_gbounce.ap().rearrange("a b -> (a b)"),
                          d_adz.ap().rearrange("a b c -> (a b c)"))

        with tc.tile_pool(name="imgp", bufs=1) as imgp:
            sb_img = imgp.tile([128, BPC * 2, D], dt.bfloat16, tag="img")
            for b in range(BPC):
                for pt in range(2):
                    nc.sync.dma_start(sb_img[:, 2 * b + pt, :],
                                      d_img[b, 128 * pt:128 * (pt + 1), :])

            # ---------------- phase A: att1w + emb_pre (to DRAM) ----------------
            with tc.tile_pool(name="phA", bufs=1) as phA, \
                 tc.tile_pool(name="phA2", bufs=2) as phA2, \
                 tc.tile_pool(name="phAw", bufs=1) as phAw, \
                 tc.tile_pool(name="phAps", bufs=2, space="PSUM") as phAps:
                sb_encw = phAw.tile([128, KD, A], dt.bfloat16, tag="bigw")
                nc.sync.dma_start(sb_encw[:], d_encw.ap().rearrange("(k p) a -> p k a", p=128, k=KD))
                sb_a1b = phAw.tile([128, KA], dt.float32, tag="a1b")
                nc.sync.dma_start(sb_a1b[:], d_att1b.ap().rearrange("(k p) -> p k", p=128))

                for b in range(BPC):
                    imgT = phA.tile([128, KD, PP], dt.bfloat16, tag="imgT")
                    nc.sync.dma_start_transpose(imgT[:], d_img[b, :, :])
                    for at in range(KA):
                        ps = phAps.tile([128, P], dt.float32, tag="a1ps")
                        for kt in range(KD):
                            nc.tensor.matmul(ps[:], sb_encw[:, kt, 128 * at:128 * (at + 1)],
                                             imgT[:, kt, :P], start=(kt == 0), stop=(kt == KD - 1))
                        ev = phA2.tile([128, P], dt.bfloat16, tag="a1ev")
                        nc.scalar.activation(ev[:], ps[:], AF.Identity,
                                             bias=sb_a1b[:, at:at + 1])
                        nc.sync.dma_start(d_att1w[b, at], ev[:])

                sb_bg = phAw.tile([128, KD], dt.float32, tag="bg")
                nc.sync.dma_start(sb_bg[:], d_bg.ap().rearrange("(m p) -> p m", p=128))
                sb_wihe = phAw.tile([128, KH, NG], dt.bfloat16, tag="bigw")
                nc.sync.dma_start(sb_wihe[:], d_wihe.ap().rearrange("(k p) g -> p k g", p=128, k=KH))
                sb_embT = phA.tile([128, KH, T * BPC], dt.bfloat16, tag="imgT")
                nc.sync.dma_start(sb_embT[:], d_embT.ap().rearrange("(k p) x -> p k x", p=128, k=KH))
                NCH = T * BPC // 2
                for gt in range(KD):
                    for hf in range(2):
                        ps = phAps.tile([128, NCH], dt.float32, tag="embps")
                        cols = slice(hf * NCH, (hf + 1) * NCH)
                        for kt in range(KH):
                            nc.tensor.matmul(ps[:], sb_wihe[:, kt, 128 * gt:128 * (gt + 1)],
                                             sb_embT[:, kt, cols], start=(kt == 0), stop=(kt == KH - 1))
                        ev = phA2.tile([128, NCH], dt.bfloat16, tag="embev")
                        nc.scalar.activation(ev[:], ps[:], AF.Identity, bias=sb_bg[:, gt:gt + 1])
                        nc.sync.dma_start(d_embpre[gt, :, cols], ev[:])

            # ---------------- scan ----------------
            ctx2 = ExitStack()
            sc = ctx2.enter_context(tc.tile_pool(name="sc", bufs=2))
            row = ctx2.enter_context(tc.tile_pool(name="row", bufs=2))
            a1p = ctx2.enter_context(tc.tile_pool(name="a1p", bufs=2))
            scw = ctx2.enter_context(tc.tile_pool(name="scw", bufs=2))
            ps_big = ctx2.enter_context(tc.tile_pool(name="psb", bufs=3, space="PSUM"))
            ps_sm = ctx2.enter_context(tc.tile_pool(name="pss", bufs=2, space="PSUM"))

            W2r = d_W2.ap().rearrange("(k p) g -> p k g", p=128, k=KD)

            for t in range(steps):
                n = ns[t]
                embp = sc.tile([128, KD, BPC], dt.bfloat16, tag="embp")
                nc.sync.dma_start(embp[:, :, :n],
                                  d_embpre[:, :, t * BPC:t * BPC + n].rearrange("k p x -> p k x"))

                # att2wT = dec_att_wf.T @ h   [128,(KA,n)]
                ps_a2 = ps_sm.tile([128, KA, BPC], dt.float32, tag="sm")
                for at in range(KA):
                    for kt in range(KH):
                        nc.tensor.matmul(ps_a2[:, at, :n], sb_decw[:, kt, 128 * at:128 * (at + 1)],
                                         sb_hTb[:, kt, :n], start=(kt == 0), stop=(kt == KH - 1))
                a2 = sc.tile([128, KA, BPC], dt.float32, tag="a2")
                nc.scalar.copy(a2[:, :, :n], ps_a2[:, :, :n])

                # score rows (batch-major in PSUM via signs_diag)
                ps_sc = ps_sm.tile([BPC, P], dt.float32, tag="sm")
                mm = 0
                for b in range(n):
                    a1 = a1p.tile([128, KA, P], dt.bfloat16, tag="a1")
                    nc.sync.dma_start(a1[:], d_att1w[b].rearrange("a p q -> p a q"))
                    ys = a1p.tile([128, KA, P], dt.bfloat16, tag="ys")
                    for at in range(KA):
                        nc.vector.tensor_scalar(ys[:, at, :], a1[:, at, :],
                                                a2[:, at, b:b + 1], 0.0,
                                                op0=OP.add, op1=OP.max)
                    for at in range(KA):
                        nc.tensor.matmul(ps_sc[:], sb_sdiag[:, at, b, :], ys[:, at, :],
                                         start=(mm == 0), stop=(mm == 4 * n - 1))
                        mm += 1

                # softmax over p
                mx = sc.tile([BPC, 1], dt.float32, tag="mx")
                nc.vector.tensor_reduce(mx[:n], ps_sc[:n], axis=AX.X, op=OP.max)
                nmx = sc.tile([BPC, 1], dt.float32, tag="nmx")
                nc.vector.tensor_scalar(nmx[:n], mx[:n], -1.0, None, op0=OP.mult)
                ex = sc.tile([BPC, PP], dt.float32, tag="ex")
                nc.scalar.activation(ex[:n, :P], ps_sc[:n], AF.Exp, bias=nmx[:n])
                nc.vector.memset(ex[:n, P:], 0.0)
                sm = sc.tile([BPC, 1], dt.float32, tag="smx")
                nc.vector.tensor_reduce(sm[:n], ex[:n, :P], axis=AX.X, op=OP.add)
                rz = sc.tile([BPC, 1], dt.float32, tag="rz")
                nc.vector.reciprocal(rz[:n], sm[:n])
                al = sc.tile([BPC, P], dt.float32, tag="al")
                nc.vector.tensor_scalar(al[:n], ex[:n, :P], rz[:n], None, op0=OP.mult)
                nc.sync.dma_start(d_alphas[t, :n, :], al[:n])
                exb = sc.tile([BPC, PP], dt.bfloat16, tag="exb")
                nc.vector.tensor_scalar(exb[:n], ex[:n], rz[:n], None, op0=OP.mult)
                # block-diagonal scatter: adiag[2b+h, b, :] = exb[b, 128h:128(h+1)]
                diag_out = bass.AP(d_adiag, 0, [[2 * BPC * 128 + 128, n], [BPC * 128, 2], [1, 128]])
                nc.sync.dma_start(diag_out, exb[:n].rearrange("b (h p) -> b h p", p=128))
                nc.sync.dma_start_transpose(
                    sb_Ablk[:, :2 * n, :].rearrange("p a b -> p (a b)"),
                    d_adiag.ap().rearrange("k c p -> (k c) p")[:2 * n * BPC, :])

                # ctx (unnormalized) = Ablk.T @ img
                ps_ctx = [ps_big.tile([BPC, D // 2], dt.float32, tag="big", name=f"ps_ctx{t}_{i}") for i in range(2)]
                for hf in range(2):
                    for chunk in range(2):
                        cl = slice(hf * 1024 + chunk * 512, hf * 1024 + (chunk + 1) * 512)
                        ocl = slice(chunk * 512, (chunk + 1) * 512)
                        for kt in range(2 * n):
                            nc.tensor.matmul(ps_ctx[hf][:n, ocl], sb_Ablk[:, kt, :n],
                                             sb_img[:, kt, cl],
                                             start=(kt == 0), stop=(kt == 2 * n - 1))

                # gate preact (f_beta_aug, ones-row bias) + sigmoid
                ps_g = [ps_big.tile([BPC, D // 2], dt.float32, tag="big", name=f"ps_g{t}_{i}") for i in range(2)]
                for hf in range(2):
                    for chunk in range(2):
                        cl = slice(hf * 1024 + chunk * 512, hf * 1024 + (chunk + 1) * 512)
                        ocl = slice(chunk * 512, (chunk + 1) * 512)
                        for kt in range(KH + 1):
                            kp = 128 if kt < KH else 1
                            nc.tensor.matmul(ps_g[hf][:n, ocl], sb_hTb[:kp, kt, :n],
                                             sb_fbe[:kp, kt, cl], start=(kt == 0), stop=(kt == KH))
                g_row = row.tile([32, NG], dt.bfloat16, tag="rowb")
                y_row = row.tile([32, NG], dt.bfloat16, tag="rowb")
                for hf in range(2):
                    hcl = slice(hf * 1024, (hf + 1) * 1024)
                    nc.scalar.activation(g_row[:n, hcl], ps_g[hf][:n], AF.Sigmoid)
                for hf in range(2):
                    hcl = slice(hf * 1024, (hf + 1) * 1024)
                    nc.vector.tensor_mul(y_row[:n, hcl], ps_ctx[hf][:n], g_row[:n, hcl])
                nc.sync.dma_start(d_ybounce.ap()[:n], y_row[:n])
                yT = sc.tile([128, KD, 32], dt.bfloat16, tag="yT")
                nc.sync.dma_start_transpose(yT[:], d_ybounce.ap())

                # gates = yT.T @ W2 (streamed) + hT.T @ w_hh_aug
                ps_gt = [ps_big.tile([BPC, NG // 2], dt.float32, tag="big", name=f"ps_gt{t}_{i}") for i in range(2)]
                for hf in range(2):
                    for chunk in range(2):
                        ocl = slice(chunk * 512, (chunk + 1) * 512)
                        gcl = slice(hf * 1024 + chunk * 512, hf * 1024 + (chunk + 1) * 512)
                        for qt in range(KD // 2):
                            w2 = scw.tile([128, 2, 512], dt.bfloat16, tag="w2", name=f"w2_{t}_{hf}_{chunk}_{qt}")
                            nc.sync.dma_start(w2[:], W2r[:, 2 * qt:2 * (qt + 1), gcl])
                            for kk in range(2):
                                kt = 2 * qt + kk
                                nc.tensor.matmul(ps_gt[hf][:n, ocl], yT[:, kt, :n],
                                                 w2[:, kk, :],
                                                 start=(kt == 0), stop=False)
                    for chunk in range(2):
                        ocl = slice(chunk * 512, (chunk + 1) * 512)
                        for kt in range(KH):
                            nc.tensor.matmul(ps_gt[hf][:n, ocl], sb_hTb[:128, kt, :n],
                                             sb_whh[:, kt, hf * 1024 + chunk * 512:hf * 1024 + (chunk + 1) * 512],
                                             start=False, stop=(kt == KH - 1))
                gr = row.tile([32, NG], dt.bfloat16, tag="rowb")
                for hf in range(2):
                    nc.vector.tensor_copy(gr[:n, hf * 1024:(hf + 1) * 1024], ps_gt[hf][:n])
                nc.sync.dma_start(d_gbounce.ap()[:n], gr[:n])
                gT = sc.tile([128, KD, 32], dt.bfloat16, tag="gT")
                nc.sync.dma_start_transpose(gT[:], d_gbounce.ap())
                gTe = sc.tile([128, KD, BPC], dt.float32, tag="gTe")
                nc.vector.tensor_add(gTe[:, :, :n], gT[:, :, :n], embp[:, :, :n])

                # LSTM pointwise (d-major)
                sig_i = sc.tile([128, KH, BPC], dt.float32, tag="sig_i")
                nc.scalar.activation(sig_i[:, :, :n], gTe[:, 0:KH, :n], AF.Sigmoid)
                sig_f = sc.tile([128, KH, BPC], dt.float32, tag="sig_f")
                nc.scalar.activation(sig_f[:, :, :n], gTe[:, KH:2 * KH, :n], AF.Sigmoid)
                tan_g = sc.tile([128, KH, BPC], dt.float32, tag="tan_g")
                nc.scalar.activation(tan_g[:, :, :n], gTe[:, 2 * KH:3 * KH, :n], AF.Tanh)
                sig_o = sc.tile([128, KH, BPC], dt.float32, tag="sig_o")
                nc.scalar.activation(sig_o[:, :, :n], gTe[:, 3 * KH:4 * KH, :n], AF.Sigmoid)
                t1 = sc.tile([128, KH, BPC], dt.float32, tag="t1")
                nc.vector.tensor_mul(t1[:, :, :n], sig_f[:, :, :n], sb_cT[:, :, :n])
                t2 = sc.tile([128, KH, BPC], dt.float32, tag="t2")
                nc.vector.tensor_mul(t2[:, :, :n], sig_i[:, :, :n], tan_g[:, :, :n])
                nc.vector.tensor_add(sb_cT[:, :, :n], t1[:, :, :n], t2[:, :, :n])
                tc_ = sc.tile([128, KH, BPC], dt.float32, tag="tc_")
                nc.scalar.activation(tc_[:, :, :n], sb_cT[:, :, :n], AF.Tanh)
                nc.vector.tensor_mul(sb_hT[:, :, :n], sig_o[:, :, :n], tc_[:, :, :n])
                nc.vector.tensor_copy(sb_hTb[:, :KH, :n], sb_hT[:, :, :n])
                nc.vector.tensor_copy(sb_hist[:, :, int(roff[t]):int(roff[t]) + n], sb_hTb[:, :KH, :n])
            ctx2.close()

        # ---------------- FC ----------------
        with tc.tile_pool(name="fcp", bufs=3) as fcp, \
             tc.tile_pool(name="fcps", bufs=4, space="PSUM") as fcps:
            sb_fcb = fcp.tile([128, VT], dt.float32, tag="fcb")
            nc.sync.dma_start(sb_fcb[:], d_fcb.ap().rearrange("(m p) -> p m", p=128))
            FCr = d_fcw.ap().rearrange("(k p) v -> p k v", p=128, k=KH)
            for mt in range(VT):
                fw = fcp.tile([128, KH, 128], dt.bfloat16, tag="fw")
                nc.sync.dma_start(fw[:], FCr[:, :, 128 * mt:128 * (mt + 1)])
                ps = fcps.tile([128, R], dt.float32, tag="fps")
                for kt in range(KH):
                    nc.tensor.matmul(ps[:], fw[:, kt, :], sb_hist[:, kt, :],
                                     start=(kt == 0), stop=(kt == KH - 1))
                ev = fcp.tile([128, R], dt.float32, tag="fev")
                if mt % 2 == 0:
                    nc.scalar.activation(ev[:], ps[:], AF.Identity, bias=sb_fcb[:, mt:mt + 1])
                else:
                    nc.vector.tensor_scalar(ev[:], ps[:], sb_fcb[:, mt:mt + 1], None, op0=OP.add)
                nc.sync.dma_start(d_predsT[128 * mt:128 * (mt + 1), :], ev[:])

    nc.compile()
    return nc


def kernel(**inputs):
    from concourse.bass_utils import run_bass_kernel_spmd

    shared, per_core, meta = _host_prep(inputs)
    ns, roff, R = meta["ns"], meta["roff"], meta["R"]

    key = (tuple(ns), R)
    if key not in _CACHE:
        _CACHE[key] = _build_program(ns, roff, R)
    nc = _CACHE[key]

    in_maps = []
    for c in range(NCORES):
        m = dict(shared)
        m.update(per_core[c])
        in_maps.append(m)

    res = run_bass_kernel_spmd(nc, in_maps, core_ids=list(range(NCORES)))
    kernel.last_results = res

    order, caps, dec_len = meta["order"], meta["caps"], meta["dec_len"]
    predictions = np.zeros((B, T, V), np.float32)
    alphas = np.zeros((B, T, P), np.float32)
    for c in range(NCORES):
        predsT = np.asarray(res.results[c]["predsT"])
        al = np.asarray(res.results[c]["alphas"])
        dl = dec_len[c::NCORES]
        for t in range(T):
            nct = min(int((dl > t).sum()), ns[t])
            if nct == 0:
                continue
            gb = c + NCORES * np.arange(nct)
            predictions[gb, t, :] = predsT[:V, int(roff[t]):int(roff[t]) + nct].T
            alphas[gb, t, :] = al[t, :nct, :]

    return predictions, caps, dec_len, alphas, order


# revision 11
# speedup vs baseline: 1.0282x; 1.0282x over previous
# kernel.py — "Show, Attend and Tell" decoder on 8 trn2 NeuronCores.
# Batch-sharded SPMD: core c owns sorted sequences c, c+8, ..., c+120 (16 each);
# no cross-core traffic. Host does index/sort/gather glue; device does the heavy
# compute: att1 projection, 31-step attention-LSTM scan, vocab FC.
#
# Key device-side tricks:
#  * full_att_w folded into enc/dec attention weights (|w| into the projections,
#    sign into the A-reduction matmul via a host-built block-diagonal constant),
#    so score = sum_a sign_a * max((att1w+att2w)_a, 0) needs one fused
#    tensor_scalar(add,max) per (seq, a-tile) and a PE reduction that lands
#    batch-major in PSUM.
#  * softmax normalization deferred: PE contracts unnormalized exp(score-max)
#    against image values; 1/Z applied as a per-partition scalar afterwards.
#  * alpha placed into a zero-padded DRAM "block diagonal" with one plain DMA,
#    then transpose-loaded so ctx for all active seqs is a single dense
#    accumulation over packed (seq, p-half) K-tiles.
#  * LSTM biases folded in via an ones-row appended to h^T (K=1 matmul rows).
#  * h kept fp32 + bf16 mirror; gates/y transposed b-major<->d-major via
#    DRAM-bounce transpose DMAs (xbar).
import math
from contextlib import ExitStack

import numpy as np
import ml_dtypes

B, L, GRID, D, P = 128, 32, 14, 2048, 196
A, E, H, V = 512, 512, 512, 10000
T = L - 1
NCORES = 8
BPC = B // NCORES
PP = 256
BF16 = ml_dtypes.bfloat16
KD, KA, KH = D // 128, A // 128, H // 128
NG = 4 * H
VT = (V + 127) // 128

_CACHE = {}


def _host_prep(inputs):
    lengths = np.asarray(inputs["lengths"])
    captions = np.asarray(inputs["captions"])
    img = np.asarray(inputs["image_embeddings"], dtype=np.float32).reshape(B, P, D)

    order = np.argsort(-lengths, kind="stable").astype(np.int32)
    lengths_s = np.asarray(lengths)[order]
    caps = captions[order]
    dec_len = (lengths_s - 1).astype(np.int32)
    img_s = img[order]

    emb = np.asarray(inputs["emb_table"], dtype=np.float32)[caps]
    mean_img = img_s.mean(axis=1)
    h0 = mean_img @ np.asarray(inputs["init_h_w"], np.float32) + np.asarray(inputs["init_h_b"], np.float32)
    c0 = mean_img @ np.asarray(inputs["init_c_w"], np.float32) + np.asarray(inputs["init_c_b"], np.float32)

    wcol = np.asarray(inputs["full_att_w"], np.float32)[:, 0]
    wabs = np.abs(wcol)
    signs = np.sign(wcol).astype(np.float32)
    enc_att_wf = np.asarray(inputs["enc_att_w"], np.float32) * wabs[None, :]
    att1_bias = (np.asarray(inputs["enc_att_b"], np.float32)
                 + np.asarray(inputs["dec_att_b"], np.float32)) * wabs
    dec_att_wf = np.asarray(inputs["dec_att_w"], np.float32) * wabs[None, :]

    signs_diag = np.zeros((128, KA, BPC, BPC), np.float32)
    for b in range(BPC):
        signs_diag[:, :, b, b] = signs.reshape(KA, 128).T

    w_ih = np.asarray(inputs["w_ih"], np.float32)
    w_ih_e = w_ih[:E, :]
    W2 = w_ih[E:, :]
    b_gates = np.asarray(inputs["b_ih"], np.float32) + np.asarray(inputs["b_hh"], np.float32)
    f_beta_aug = np.zeros((H + 128, D), np.float32)
    f_beta_aug[:H] = np.asarray(inputs["f_beta_w"], np.float32)
    f_beta_aug[H] = np.asarray(inputs["f_beta_b"], np.float32)

    fc_wp = np.zeros((H, VT * 128), np.float32)
    fc_wp[:, :V] = np.asarray(inputs["fc_w"], np.float32)
    fc_bp = np.zeros((VT * 128,), np.float32)
    fc_bp[:V] = np.asarray(inputs["fc_b"], np.float32)

    n_t = [int((dec_len > t).sum()) for t in range(T)]
    ns = [max(1, min(BPC, math.ceil(nt / NCORES))) for nt in n_t]
    roff = np.cumsum([0] + ns).astype(np.int64)
    R = int(roff[-1])

    per_core = []
    for c in range(NCORES):
        idx = np.arange(c, B, NCORES)
        img_pad = np.zeros((BPC, PP, D), dtype=BF16)
        img_pad[:, :P, :] = img_s[idx].astype(BF16)
        embT = np.ascontiguousarray(
            emb[idx][:, :T, :].transpose(2, 1, 0).reshape(E, T * BPC)).astype(BF16)
        per_core.append({
            "img": img_pad,
            "embT": embT,
            "h0T": np.ascontiguousarray(h0[idx].T).astype(np.float32),
            "c0T": np.ascontiguousarray(c0[idx].T).astype(np.float32),
        })

    shared = {
        "enc_att_wf": enc_att_wf.astype(BF16),
        "att1_bias": att1_bias.astype(np.float32),
        "dec_att_wf": dec_att_wf.astype(BF16),
        "signs_diag": signs_diag.astype(BF16),
        "w_ih_e": w_ih_e.astype(BF16),
        "W2": W2.astype(BF16),
        "w_hh": np.asarray(inputs["w_hh"], np.float32).astype(BF16),
        "b_gates": b_gates.astype(np.float32),
        "f_beta_aug": f_beta_aug.astype(BF16),
        "fc_w": fc_wp.astype(BF16),
        "fc_b": fc_bp.astype(np.float32),
        "alpha_diag_zero": np.zeros((2 * BPC, BPC, 128), BF16),
    }
    meta = {"order": order, "caps": caps, "dec_len": dec_len,
            "ns": ns, "roff": roff, "R": R}
    return shared, per_core, meta


def _build_program(ns, roff, R):
    import concourse.bass as bass
    import concourse.mybir as mybir
    import concourse.tile as tile
    from concourse import bacc
    from concourse import tile_utils
    tile_utils.max_sbuf_usage = 212 * 1024

    dt = mybir.dt
    AF = mybir.ActivationFunctionType
    OP = mybir.AluOpType
    AX = mybir.AxisListType

    nc = bacc.Bacc("TRN2", target_bir_lowering=False, debug=False,
                   enable_asserts=False, num_devices=NCORES)

    d_img = nc.dram_tensor("img", [BPC, PP, D], dt.bfloat16, kind="ExternalInput")
    d_embT = nc.dram_tensor("embT", [E, T * BPC], dt.bfloat16, kind="ExternalInput")
    d_h0T = nc.dram_tensor("h0T", [H, BPC], dt.float32, kind="ExternalInput")
    d_c0T = nc.dram_tensor("c0T", [H, BPC], dt.float32, kind="ExternalInput")
    d_encw = nc.dram_tensor("enc_att_wf", [D, A], dt.bfloat16, kind="ExternalInput")
    d_att1b = nc.dram_tensor("att1_bias", [A], dt.float32, kind="ExternalInput")
    d_decw = nc.dram_tensor("dec_att_wf", [H, A], dt.bfloat16, kind="ExternalInput")
    d_sdiag = nc.dram_tensor("signs_diag", [128, KA, BPC, BPC], dt.bfloat16, kind="ExternalInput")
    d_wihe = nc.dram_tensor("w_ih_e", [E, NG], dt.bfloat16, kind="ExternalInput")
    d_W2 = nc.dram_tensor("W2", [D, NG], dt.bfloat16, kind="ExternalInput")
    d_whh = nc.dram_tensor("w_hh", [H, NG], dt.bfloat16, kind="ExternalInput")
    d_bg = nc.dram_tensor("b_gates", [NG], dt.float32, kind="ExternalInput")
    d_fbeta = nc.dram_tensor("f_beta_aug", [H + 128, D], dt.bfloat16, kind="ExternalInput")
    d_fcw = nc.dram_tensor("fc_w", [H, VT * 128], dt.bfloat16, kind="ExternalInput")
    d_fcb = nc.dram_tensor("fc_b", [VT * 128], dt.float32, kind="ExternalInput")
    d_adz = nc.dram_tensor("alpha_diag_zero", [2 * BPC, BPC, 128], dt.bfloat16, kind="ExternalInput")

    d_predsT = nc.dram_tensor("predsT", [VT * 128, R], dt.float32, kind="ExternalOutput")
    d_alphas = nc.dram_tensor("alphas", [T, BPC, P], dt.float32, kind="ExternalOutput")

    d_att1w = nc.dram_tensor("att1w_scr", [BPC, KA, 128, P], dt.bfloat16, kind="Internal")
    d_embpre = nc.dram_tensor("embpre_scr", [KD, 128, T * BPC], dt.bfloat16, kind="Internal")
    d_adiag = nc.dram_tensor("alpha_diag", [2 * BPC, BPC, 128], dt.bfloat16, kind="Internal")
    d_ybounce = nc.dram_tensor("y_bounce", [32, NG], dt.bfloat16, kind="Internal")
    d_gbounce = nc.dram_tensor("g_bounce", [32, NG], dt.bfloat16, kind="Internal")

    steps = len(ns)

    with tile.TileContext(nc) as tc, ExitStack() as ctx:
        pers = ctx.enter_context(tc.tile_pool(name="pers", bufs=1))
        sb_hT = pers.tile([128, KH, BPC], dt.float32, tag="hT")
        sb_hTb = pers.tile([128, KH + 1, BPC], dt.bfloat16, tag="hTb")
        sb_cT = pers.tile([128, KH, BPC], dt.float32, tag="cT")
        sb_hist = pers.tile([128, KH, R], dt.bfloat16, tag="hist")
        sb_decw = pers.tile([128, KH, A], dt.bfloat16, tag="decw")
        sb_sdiag = pers.tile([128, KA, BPC, BPC], dt.bfloat16, tag="sdiag")
        sb_whh = pers.tile([128, KH, NG], dt.bfloat16, tag="whh")
        sb_fbe = pers.tile([128, KH + 1, D], dt.bfloat16, tag="fbe")
        sb_Ablk = pers.tile([128, 2 * BPC, BPC], dt.bfloat16, tag="Ablk")

        nc.sync.dma_start(sb_decw[:], d_decw.ap().rearrange("(k p) a -> p k a", p=128))
        nc.sync.dma_start(sb_sdiag[:], d_sdiag.ap())
        nc.sync.dma_start(sb_whh[:], d_whh.ap().rearrange("(k p) g -> p k g", p=128, k=KH))
        nc.sync.dma_start(sb_fbe[:], d_fbeta.ap().rearrange("(k p) g -> p k g", p=128, k=KH + 1))
        nc.sync.dma_start(sb_hT[:], d_h0T.ap().rearrange("(k p) b -> p k b", p=128))
        nc.sync.dma_start(sb_cT[:], d_c0T.ap().rearrange("(k p) b -> p k b", p=128))
        nc.vector.memset(sb_hTb[:, KH, :], 0.0)
        nc.vector.memset(sb_hTb[:1, KH, :], 1.0)
        nc.vector.tensor_copy(sb_hTb[:, :KH, :], sb_hT[:])
        nc.sync.dma_start(d_adiag.ap(), d_adz.ap())
        nc.sync.dma_start(d_ybounce.ap().rearrange("a b -> (a b)"),
                          d_adz.ap().rearrange("a b c -> (a b c)"))
        nc.sync.dma_start(d_gbounce.ap().rearrange("a b -> (a b)"),
                          d_adz.ap().rearrange("a b c -> (a b c)"))

        with tc.tile_pool(name="imgp", bufs=1) as imgp:
            sb_img = imgp.tile([128, BPC * 2, D], dt.bfloat16, tag="img")
            for b in range(BPC):
                for pt in range(2):
                    nc.sync.dma_start(sb_img[:, 2 * b + pt, :],
                                      d_img[b, 128 * pt:128 * (pt + 1), :])

            # ---------------- phase A: att1w + emb_pre (to DRAM) ----------------
            with tc.tile_pool(name="phA", bufs=1) as phA, \
                 tc.tile_pool(name="phA2", bufs=2) as phA2, \
                 tc.tile_pool(name="phAw", bufs=1) as phAw, \
                 tc.tile_pool(name="phAps", bufs=2, space="PSUM") as phAps:
                sb_encw = phAw.tile([128, KD, A], dt.bfloat16, tag="bigw")
                nc.sync.dma_start(sb_encw[:], d_encw.ap().rearrange("(k p) a -> p k a", p=128, k=KD))
                sb_a1b = phAw.tile([128, KA], dt.float32, tag="a1b")
                nc.sync.dma_start(sb_a1b[:], d_att1b.ap().rearrange("(k p) -> p k", p=128))

                for b in range(BPC):
                    imgT = phA.tile([128, KD, PP], dt.bfloat16, tag="imgT")
                    nc.sync.dma_start_transpose(imgT[:], d_img[b, :, :])
                    for at in range(KA):
                        ps = phAps.tile([128, P], dt.float32, tag="a1ps")
                        for kt in range(KD):
                            nc.tensor.matmul(ps[:], sb_encw[:, kt, 128 * at:128 * (at + 1)],
                                             imgT[:, kt, :P], start=(kt == 0), stop=(kt == KD - 1))
                        ev = phA2.tile([128, P], dt.bfloat16, tag="a1ev")
                        nc.scalar.activation(ev[:], ps[:], AF.Identity,
                                             bias=sb_a1b[:, at:at + 1])
                        nc.sync.dma_start(d_att1w[b, at], ev[:])

                sb_bg = phAw.tile([128, KD], dt.float32, tag="bg")
                nc.sync.dma_start(sb_bg[:], d_bg.ap().rearrange("(m p) -> p m", p=128))
                sb_wihe = phAw.tile([128, KH, NG], dt.bfloat16, tag="bigw")
                nc.sync.dma_start(sb_wihe[:], d_wihe.ap().rearrange("(k p) g -> p k g", p=128, k=KH))
                sb_embT = phA.tile([128, KH, T * BPC], dt.bfloat16, tag="imgT")
                nc.sync.dma_start(sb_embT[:], d_embT.ap().rearrange("(k p) x -> p k x", p=128, k=KH))
                NCH = T * BPC // 2
                for gt in range(KD):
                    for hf in range(2):
                        ps = phAps.tile([128, NCH], dt.float32, tag="embps")
                        cols = slice(hf * NCH, (hf + 1) * NCH)
                        for kt in range(KH):
                            nc.tensor.matmul(ps[:], sb_wihe[:, kt, 128 * gt:128 * (gt + 1)],
                                             sb_embT[:, kt, cols], start=(kt == 0), stop=(kt == KH - 1))
                        ev = phA2.tile([128, NCH], dt.bfloat16, tag="embev")
                        nc.scalar.activation(ev[:], ps[:], AF.Identity, bias=sb_bg[:, gt:gt + 1])
                        nc.sync.dma_start(d_embpre[gt, :, cols], ev[:])

            # ---------------- scan ----------------
            ctx2 = ExitStack()
            sc = ctx2.enter_context(tc.tile_pool(name="sc", bufs=2))
            row = ctx2.enter_context(tc.tile_pool(name="row", bufs=2))
            a1p = ctx2.enter_context(tc.tile_pool(name="a1p", bufs=2))
            scw = ctx2.enter_context(tc.tile_pool(name="scw", bufs=2))
            ps_big = ctx2.enter_context(tc.tile_pool(name="psb", bufs=3, space="PSUM"))
            ps_sm = ctx2.enter_context(tc.tile_pool(name="pss", bufs=2, space="PSUM"))

            W2r = d_W2.ap().rearrange("(k p) g -> p k g", p=128, k=KD)

            for t in range(steps):
                n = ns[t]
                embp = sc.tile([128, KD, BPC], dt.bfloat16, tag="embp")
                nc.sync.dma_start(embp[:, :, :n],
                                  d_embpre[:, :, t * BPC:t * BPC + n].rearrange("k p x -> p k x"))

                # att2wT = dec_att_wf.T @ h   [128,(KA,n)]
                ps_a2 = ps_sm.tile([128, KA, BPC], dt.float32, tag="sm")
                for at in range(KA):
                    for kt in range(KH):
                        nc.tensor.matmul(ps_a2[:, at, :n], sb_decw[:, kt, 128 * at:128 * (at + 1)],
                                         sb_hTb[:, kt, :n], start=(kt == 0), stop=(kt == KH - 1))
                a2 = sc.tile([128, KA, BPC], dt.float32, tag="a2")
                nc.scalar.copy(a2[:, :, :n], ps_a2[:, :, :n])

                # gate preact (f_beta_aug ones-row bias) early: only needs h
                ps_g = [ps_big.tile([BPC, D // 2], dt.float32, tag="big", name=f"ps_g{t}_{i}") for i in range(2)]
                for hf in range(2):
                    for chunk in range(2):
                        cl = slice(hf * 1024 + chunk * 512, hf * 1024 + (chunk + 1) * 512)
                        ocl = slice(chunk * 512, (chunk + 1) * 512)
                        for kt in range(KH + 1):
                            kp = 128 if kt < KH else 1
                            nc.tensor.matmul(ps_g[hf][:n, ocl], sb_hTb[:kp, kt, :n],
                                             sb_fbe[:kp, kt, cl], start=(kt == 0), stop=(kt == KH))
                th_row = row.tile([32, NG], dt.bfloat16, tag="rowb")
                for hf in range(2):
                    hcl = slice(hf * 1024, (hf + 1) * 1024)
                    nc.scalar.activation(th_row[:n, hcl], ps_g[hf][:n], AF.Tanh, scale=0.5)
                g_row = row.tile([32, NG], dt.bfloat16, tag="rowb")
                nc.vector.tensor_scalar(g_row[:n], th_row[:n], 0.5, 0.5, op0=OP.mult, op1=OP.add)

                # score rows (batch-major in PSUM via signs_diag)
                ps_sc = ps_sm.tile([BPC, P], dt.float32, tag="sm")
                mm = 0
                for b in range(n):
                    a1 = a1p.tile([128, KA, P], dt.bfloat16, tag="a1")
                    nc.sync.dma_start(a1[:], d_att1w[b].rearrange("a p q -> p a q"))
                    ys = a1p.tile([128, KA, P], dt.bfloat16, tag="ys")
                    for at in range(KA):
                        nc.vector.tensor_scalar(ys[:, at, :], a1[:, at, :],
                                                a2[:, at, b:b + 1], 0.0,
                                                op0=OP.add, op1=OP.max)
                    for at in range(KA):
                        nc.tensor.matmul(ps_sc[:], sb_sdiag[:, at, b, :], ys[:, at, :],
                                         start=(mm == 0), stop=(mm == 4 * n - 1))
                        mm += 1

                # softmax over p
                mx = sc.tile([BPC, 1], dt.float32, tag="mx")
                nc.vector.tensor_reduce(mx[:n], ps_sc[:n], axis=AX.X, op=OP.max)
                nmx = sc.tile([BPC, 1], dt.float32, tag="nmx")
                nc.vector.tensor_scalar(nmx[:n], mx[:n], -1.0, None, op0=OP.mult)
                ex = sc.tile([BPC, PP], dt.float32, tag="ex")
                nc.scalar.activation(ex[:n, :P], ps_sc[:n], AF.Exp, bias=nmx[:n])
                nc.vector.memset(ex[:n, P:], 0.0)
                sm = sc.tile([BPC, 1], dt.float32, tag="smx")
                nc.vector.tensor_reduce(sm[:n], ex[:n, :P], axis=AX.X, op=OP.add)
                rz = sc.tile([BPC, 1], dt.float32, tag="rz")
                nc.vector.reciprocal(rz[:n], sm[:n])
                al = sc.tile([BPC, P], dt.float32, tag="al")
                nc.vector.tensor_scalar(al[:n], ex[:n, :P], rz[:n], None, op0=OP.mult)
                nc.sync.dma_start(d_alphas[t, :n, :], al[:n])
                exb = sc.tile([BPC, PP], dt.bfloat16, tag="exb")
                nc.vector.tensor_scalar(exb[:n], ex[:n], rz[:n], None, op0=OP.mult)
                # block-diagonal scatter: adiag[2b+h, b, :] = exb[b, 128h:128(h+1)]
                diag_out = bass.AP(d_adiag, 0, [[2 * BPC * 128 + 128, n], [BPC * 128, 2], [1, 128]])
                nc.sync.dma_start(diag_out, exb[:n].rearrange("b (h p) -> b h p", p=128))
                nc.sync.dma_start_transpose(
                    sb_Ablk[:, :2 * n, :].rearrange("p a b -> p (a b)"),
                    d_adiag.ap().rearrange("k c p -> (k c) p")[:2 * n * BPC, :])

                # ctx (unnormalized) = Ablk.T @ img
                ps_ctx = [ps_big.tile([BPC, D // 2], dt.float32, tag="big", name=f"ps_ctx{t}_{i}") for i in range(2)]
                for hf in range(2):
                    for chunk in range(2):
                        cl = slice(hf * 1024 + chunk * 512, hf * 1024 + (chunk + 1) * 512)
                        ocl = slice(chunk * 512, (chunk + 1) * 512)
                        for kt in range(2 * n):
                            nc.tensor.matmul(ps_ctx[hf][:n, ocl], sb_Ablk[:, kt, :n],
                                             sb_img[:, kt, cl],
                                             start=(kt == 0), stop=(kt == 2 * n - 1))

                y_row = row.tile([32, NG], dt.bfloat16, tag="rowb")
                for hf in range(2):
                    hcl = slice(hf * 1024, (hf + 1) * 1024)
                    nc.vector.tensor_mul(y_row[:n, hcl], ps_ctx[hf][:n], g_row[:n, hcl])
                nc.sync.dma_start(d_ybounce.ap()[:n], y_row[:n])
                yT = sc.tile([128, KD, 32], dt.bfloat16, tag="yT")
                nc.sync.dma_start_transpose(yT[:], d_ybounce.ap())

                # gates = yT.T @ W2 (streamed) + hT.T @ w_hh_aug
                ps_gt = [ps_big.tile([BPC, NG // 2], dt.float32, tag="big", name=f"ps_gt{t}_{i}") for i in range(2)]
                for hf in range(2):
                    for chunk in range(2):
                        ocl = slice(chunk * 512, (chunk + 1) * 512)
                        gcl = slice(hf * 1024 + chunk * 512, hf * 1024 + (chunk + 1) * 512)
                        for qt in range(KD // 2):
                            w2 = scw.tile([128, 2, 512], dt.bfloat16, tag="w2", name=f"w2_{t}_{hf}_{chunk}_{qt}")
                            nc.sync.dma_start(w2[:], W2r[:, 2 * qt:2 * (qt + 1), gcl])
                            for kk in range(2):
                                kt = 2 * qt + kk
                                nc.tensor.matmul(ps_gt[hf][:n, ocl], yT[:, kt, :n],
                                                 w2[:, kk, :],
                                                 start=(kt == 0), stop=False)
                    for chunk in range(2):
                        ocl = slice(chunk * 512, (chunk + 1) * 512)
                        for kt in range(KH):
                            nc.tensor.matmul(ps_gt[hf][:n, ocl], sb_hTb[:128, kt, :n],
                                             sb_whh[:, kt, hf * 1024 + chunk * 512:hf * 1024 + (chunk + 1) * 512],
                                             start=False, stop=(kt == KH - 1))
                gr = row.tile([32, NG], dt.bfloat16, tag="rowb")
                for hf in range(2):
                    nc.vector.tensor_copy(gr[:n, hf * 1024:(hf + 1) * 1024], ps_gt[hf][:n])
                nc.sync.dma_start(d_gbounce.ap()[:n], gr[:n])
                gT = sc.tile([128, KD, 32], dt.bfloat16, tag="gT")
                nc.sync.dma_start_transpose(gT[:], d_gbounce.ap())
                gTe = sc.tile([128, KD, BPC], dt.float32, tag="gTe")
                nc.vector.tensor_add(gTe[:, :, :n], gT[:, :, :n], embp[:, :, :n])

                # LSTM pointwise (d-major)
                sig_i = sc.tile([128, KH, BPC], dt.float32, tag="sig_i")
                nc.scalar.activation(sig_i[:, :, :n], gTe[:, 0:KH, :n], AF.Tanh, scale=0.5)
                nc.vector.tensor_scalar(sig_i[:, :, :n], sig_i[:, :, :n], 0.5, 0.5, op0=OP.mult, op1=OP.add)
                sig_f = sc.tile([128, KH, BPC], dt.float32, tag="sig_f")
                nc.scalar.activation(sig_f[:, :, :n], gTe[:, KH:2 * KH, :n], AF.Tanh, scale=0.5)
                nc.vector.tensor_scalar(sig_f[:, :, :n], sig_f[:, :, :n], 0.5, 0.5, op0=OP.mult, op1=OP.add)
                tan_g = sc.tile([128, KH, BPC], dt.float32, tag="tan_g")
                nc.scalar.activation(tan_g[:, :, :n], gTe[:, 2 * KH:3 * KH, :n], AF.Tanh)
                sig_o = sc.tile([128, KH, BPC], dt.float32, tag="sig_o")
                nc.scalar.activation(sig_o[:, :, :n], gTe[:, 3 * KH:4 * KH, :n], AF.Tanh, scale=0.5)
                nc.vector.tensor_scalar(sig_o[:, :, :n], sig_o[:, :, :n], 0.5, 0.5, op0=OP.mult, op1=OP.add)
                t1 = sc.tile([128, KH, BPC], dt.float32, tag="t1")
                nc.vector.tensor_mul(t1[:, :, :n], sig_f[:, :, :n], sb_cT[:, :, :n])
                t2 = sc.tile([128, KH, BPC], dt.float32, tag="t2")
                nc.vector.tensor_mul(t2[:, :, :n], sig_i[:, :, :n], tan_g[:, :, :n])
                nc.vector.tensor_add(sb_cT[:, :, :n], t1[:, :, :n], t2[:, :, :n])
                tc_ = sc.tile([128, KH, BPC], dt.float32, tag="tc_")
                nc.scalar.activation(tc_[:, :, :n], sb_cT[:, :, :n], AF.Tanh)
                nc.vector.tensor_mul(sb_hT[:, :, :n], sig_o[:, :, :n], tc_[:, :, :n])
                nc.vector.tensor_copy(sb_hTb[:, :KH, :n], sb_hT[:, :, :n])
                nc.vector.tensor_copy(sb_hist[:, :, int(roff[t]):int(roff[t]) + n], sb_hTb[:, :KH, :n])
            ctx2.close()

        # ---------------- FC ----------------
        with tc.tile_pool(name="fcp", bufs=3) as fcp, \
             tc.tile_pool(name="fcps", bufs=4, space="PSUM") as fcps:
            sb_fcb = fcp.tile([128, VT], dt.float32, tag="fcb")
            nc.sync.dma_start(sb_fcb[:], d_fcb.ap().rearrange("(m p) -> p m", p=128))
            FCr = d_fcw.ap().rearrange("(k p) v -> p k v", p=128, k=KH)
            for mt in range(VT):
                fw = fcp.tile([128, KH, 128], dt.bfloat16, tag="fw")
                nc.sync.dma_start(fw[:], FCr[:, :, 128 * mt:128 * (mt + 1)])
                ps = fcps.tile([128, R], dt.float32, tag="fps")
                for kt in range(KH):
                    nc.tensor.matmul(ps[:], fw[:, kt, :], sb_hist[:, kt, :],
                                     start=(kt == 0), stop=(kt == KH - 1))
                ev = fcp.tile([128, R], dt.float32, tag="fev")
                if mt % 2 == 0:
                    nc.scalar.activation(ev[:], ps[:], AF.Identity, bias=sb_fcb[:, mt:mt + 1])
                else:
                    nc.vector.tensor_scalar(ev[:], ps[:], sb_fcb[:, mt:mt + 1], None, op0=OP.add)
                nc.sync.dma_start(d_predsT[128 * mt:128 * (mt + 1), :], ev[:])

    nc.compile()
    return nc


def kernel(**inputs):
    from concourse.bass_utils import run_bass_kernel_spmd

    shared, per_core, meta = _host_prep(inputs)
    ns, roff, R = meta["ns"], meta["roff"], meta["R"]

    key = (tuple(ns), R)
    if key not in _CACHE:
        _CACHE[key] = _build_program(ns, roff, R)
    nc = _CACHE[key]

    in_maps = []
    for c in range(NCORES):
        m = dict(shared)
        m.update(per_core[c])
        in_maps.append(m)

    res = run_bass_kernel_spmd(nc, in_maps, core_ids=list(range(NCORES)))
    kernel.last_results = res

    order, caps, dec_len = meta["order"], meta["caps"], meta["dec_len"]
    predictions = np.zeros((B, T, V), np.float32)
    alphas = np.zeros((B, T, P), np.float32)
    for c in range(NCORES):
        predsT = np.asarray(res.results[c]["predsT"])
        al = np.asarray(res.results[c]["alphas"])
        dl = dec_len[c::NCORES]
        for t in range(T):
            nct = min(int((dl > t).sum()), ns[t])
            if nct == 0:
                continue
            gb = c + NCORES * np.arange(nct)
            predictions[gb, t, :] = predsT[:V, int(roff[t]):int(roff[t]) + nct].T
            alphas[gb, t, :] = al[t, :nct, :]

    return predictions, caps, dec_len, alphas, order


# revision 13
# speedup vs baseline: 1.2473x; 1.2131x over previous
# kernel.py — "Show, Attend and Tell" decoder on 8 trn2 NeuronCores.
# Batch-sharded SPMD: core c owns sorted sequences c, c+8, ..., c+120 (16 each);
# no cross-core traffic. Host does index/sort/gather glue; device does the heavy
# compute: att1 projection, 31-step attention-LSTM scan, vocab FC.
#
# Key device-side tricks:
#  * full_att_w folded into enc/dec attention weights (|w| into the projections,
#    sign into the A-reduction matmul via a host-built block-diagonal constant),
#    so score = sum_a sign_a * max((att1w+att2w)_a, 0) needs one fused
#    tensor_scalar(add,max) per (seq, a-tile) and a PE reduction that lands
#    batch-major in PSUM.
#  * softmax normalization deferred: PE contracts unnormalized exp(score-max)
#    against image values; 1/Z applied as a per-partition scalar afterwards.
#  * alpha placed into a zero-padded DRAM "block diagonal" with one plain DMA,
#    then transpose-loaded so ctx for all active seqs is a single dense
#    accumulation over packed (seq, p-half) K-tiles.
#  * LSTM biases folded in via an ones-row appended to h^T (K=1 matmul rows).
#  * h kept fp32 + bf16 mirror; gates/y transposed b-major<->d-major via
#    DRAM-bounce transpose DMAs (xbar).
import math
from contextlib import ExitStack

import numpy as np
import ml_dtypes

B, L, GRID, D, P = 128, 32, 14, 2048, 196
A, E, H, V = 512, 512, 512, 10000
T = L - 1
NCORES = 8
BPC = B // NCORES
PP = 256
BF16 = ml_dtypes.bfloat16
KD, KA, KH = D // 128, A // 128, H // 128
NG = 4 * H
VT = (V + 127) // 128

_CACHE = {}


def _host_prep(inputs):
    lengths = np.asarray(inputs["lengths"])
    captions = np.asarray(inputs["captions"])
    img = np.asarray(inputs["image_embeddings"], dtype=np.float32).reshape(B, P, D)

    order = np.argsort(-lengths, kind="stable").astype(np.int32)
    lengths_s = np.asarray(lengths)[order]
    caps = captions[order]
    dec_len = (lengths_s - 1).astype(np.int32)
    img_s = img[order]

    emb = np.asarray(inputs["emb_table"], dtype=np.float32)[caps]
    mean_img = img_s.mean(axis=1)
    h0 = mean_img @ np.asarray(inputs["init_h_w"], np.float32) + np.asarray(inputs["init_h_b"], np.float32)
    c0 = mean_img @ np.asarray(inputs["init_c_w"], np.float32) + np.asarray(inputs["init_c_b"], np.float32)

    wcol = np.asarray(inputs["full_att_w"], np.float32)[:, 0]
    wabs = np.abs(wcol)
    signs = np.sign(wcol).astype(np.float32)
    enc_att_wf = np.asarray(inputs["enc_att_w"], np.float32) * wabs[None, :]
    att1_bias = (np.asarray(inputs["enc_att_b"], np.float32)
                 + np.asarray(inputs["dec_att_b"], np.float32)) * wabs
    dec_att_wf = np.asarray(inputs["dec_att_w"], np.float32) * wabs[None, :]

    signs_diag = np.zeros((128, KA, BPC, BPC), np.float32)
    for b in range(BPC):
        signs_diag[:, :, b, b] = signs.reshape(KA, 128).T

    w_ih = np.asarray(inputs["w_ih"], np.float32)
    w_ih_e = w_ih[:E, :]
    W2 = w_ih[E:, :]
    b_gates = np.asarray(inputs["b_ih"], np.float32) + np.asarray(inputs["b_hh"], np.float32)
    f_beta_aug = np.zeros((H + 128, D), np.float32)
    f_beta_aug[:H] = np.asarray(inputs["f_beta_w"], np.float32)
    f_beta_aug[H] = np.asarray(inputs["f_beta_b"], np.float32)

    fc_wp = np.zeros((H, VT * 128), np.float32)
    fc_wp[:, :V] = np.asarray(inputs["fc_w"], np.float32)
    fc_bp = np.zeros((VT * 128,), np.float32)
    fc_bp[:V] = np.asarray(inputs["fc_b"], np.float32)

    n_t = [int((dec_len > t).sum()) for t in range(T)]
    ns = [max(1, min(BPC, math.ceil(nt / NCORES))) for nt in n_t]
    roff = np.cumsum([0] + ns).astype(np.int64)
    R = int(roff[-1])

    per_core = []
    for c in range(NCORES):
        idx = np.arange(c, B, NCORES)
        img_pad = np.zeros((BPC, PP, D), dtype=BF16)
        img_pad[:, :P, :] = img_s[idx].astype(BF16)
        embT = np.ascontiguousarray(
            emb[idx][:, :T, :].transpose(2, 1, 0).reshape(E, T * BPC)).astype(BF16)
        per_core.append({
            "img": img_pad,
            "embT": embT,
            "h0T": np.ascontiguousarray(h0[idx].T).astype(np.float32),
            "c0T": np.ascontiguousarray(c0[idx].T).astype(np.float32),
        })

    shared = {
        "enc_att_wf": np.ascontiguousarray(enc_att_wf.reshape(KD, 128, A)).astype(BF16),
        "att1_bias": att1_bias.astype(np.float32),
        "dec_att_wf": np.ascontiguousarray(dec_att_wf.reshape(KH, 128, A)).astype(BF16),
        "signs_diag": signs_diag.astype(BF16),
        "w_ih_e": np.ascontiguousarray(w_ih_e.reshape(KH, 128, NG)).astype(BF16),
        "W2": np.ascontiguousarray(W2.reshape(KD, 128, NG)).astype(BF16),
        "w_hh": np.ascontiguousarray(np.asarray(inputs["w_hh"], np.float32).reshape(KH, 128, NG)).astype(BF16),
        "b_gates": b_gates.astype(np.float32),
        "f_beta_aug": np.ascontiguousarray(f_beta_aug.reshape(KH + 1, 128, D)).astype(BF16),
        "fc_w": np.ascontiguousarray(fc_wp.reshape(KH, 128, VT, 128).transpose(2, 0, 1, 3)).astype(BF16),
        "fc_b": fc_bp.astype(np.float32),
        "alpha_diag_zero": np.zeros((2 * BPC, BPC, 128), BF16),
    }
    meta = {"order": order, "caps": caps, "dec_len": dec_len,
            "ns": ns, "roff": roff, "R": R}
    return shared, per_core, meta


def _build_program(ns, roff, R):
    import concourse.bass as bass
    import concourse.mybir as mybir
    import concourse.tile as tile
    from concourse import bacc
    from concourse import tile_utils
    tile_utils.max_sbuf_usage = 222 * 1024

    dt = mybir.dt
    AF = mybir.ActivationFunctionType
    OP = mybir.AluOpType
    AX = mybir.AxisListType

    nc = bacc.Bacc("TRN2", target_bir_lowering=False, debug=False,
                   enable_asserts=False, num_devices=NCORES)

    d_img = nc.dram_tensor("img", [BPC, PP, D], dt.bfloat16, kind="ExternalInput")
    d_embT = nc.dram_tensor("embT", [E, T * BPC], dt.bfloat16, kind="ExternalInput")
    d_h0T = nc.dram_tensor("h0T", [H, BPC], dt.float32, kind="ExternalInput")
    d_c0T = nc.dram_tensor("c0T", [H, BPC], dt.float32, kind="ExternalInput")
    d_encw = nc.dram_tensor("enc_att_wf", [KD, 128, A], dt.bfloat16, kind="ExternalInput")
    d_att1b = nc.dram_tensor("att1_bias", [A], dt.float32, kind="ExternalInput")
    d_decw = nc.dram_tensor("dec_att_wf", [KH, 128, A], dt.bfloat16, kind="ExternalInput")
    d_sdiag = nc.dram_tensor("signs_diag", [128, KA, BPC, BPC], dt.bfloat16, kind="ExternalInput")
    d_wihe = nc.dram_tensor("w_ih_e", [KH, 128, NG], dt.bfloat16, kind="ExternalInput")
    d_W2 = nc.dram_tensor("W2", [KD, 128, NG], dt.bfloat16, kind="ExternalInput")
    d_whh = nc.dram_tensor("w_hh", [KH, 128, NG], dt.bfloat16, kind="ExternalInput")
    d_bg = nc.dram_tensor("b_gates", [NG], dt.float32, kind="ExternalInput")
    d_fbeta = nc.dram_tensor("f_beta_aug", [KH + 1, 128, D], dt.bfloat16, kind="ExternalInput")
    d_fcw = nc.dram_tensor("fc_w", [VT, KH, 128, 128], dt.bfloat16, kind="ExternalInput")
    d_fcb = nc.dram_tensor("fc_b", [VT * 128], dt.float32, kind="ExternalInput")
    d_adz = nc.dram_tensor("alpha_diag_zero", [2 * BPC, BPC, 128], dt.bfloat16, kind="ExternalInput")

    d_predsT = nc.dram_tensor("predsT", [VT * 128, R], dt.float32, kind="ExternalOutput")
    d_alphas = nc.dram_tensor("alphas", [T, BPC, P], dt.float32, kind="ExternalOutput")

    d_att1w = nc.dram_tensor("att1w_scr", [BPC, 128, KA, P], dt.bfloat16, kind="Internal")
    d_embpre = nc.dram_tensor("embpre_scr", [T, 128, KD, BPC], dt.bfloat16, kind="Internal")
    d_adiag = nc.dram_tensor("alpha_diag", [2 * BPC, BPC, 128], dt.bfloat16, kind="Internal")
    d_ybounce = nc.dram_tensor("y_bounce", [32, NG], dt.bfloat16, kind="Internal")
    d_gbounce = nc.dram_tensor("g_bounce", [32, NG], dt.bfloat16, kind="Internal")

    steps = len(ns)

    with tile.TileContext(nc) as tc, ExitStack() as ctx:
        pers = ctx.enter_context(tc.tile_pool(name="pers", bufs=1))
        sb_hT = pers.tile([128, KH, BPC], dt.float32, tag="hT")
        sb_hTb = pers.tile([128, KH + 1, BPC], dt.bfloat16, tag="hTb")
        sb_cT = pers.tile([128, KH, BPC], dt.float32, tag="cT")
        sb_hist = pers.tile([128, KH, R], dt.bfloat16, tag="hist")
        sb_decw = pers.tile([128, KH, A], dt.bfloat16, tag="decw")
        sb_sdiag = pers.tile([128, KA, BPC, BPC], dt.bfloat16, tag="sdiag")
        sb_fbe = pers.tile([128, KH + 1, D], dt.bfloat16, tag="fbe")
        sb_Ablk = pers.tile([128, 2 * BPC, BPC], dt.bfloat16, tag="Ablk")

        nc.sync.dma_start(sb_decw[:], d_decw.ap().rearrange("k p a -> p k a"))
        nc.sync.dma_start(sb_sdiag[:], d_sdiag.ap())
        nc.sync.dma_start(sb_fbe[:], d_fbeta.ap().rearrange("k p g -> p k g"))
        nc.sync.dma_start(sb_hT[:], d_h0T.ap().rearrange("(k p) b -> p k b", p=128))
        nc.sync.dma_start(sb_cT[:], d_c0T.ap().rearrange("(k p) b -> p k b", p=128))
        nc.vector.memset(sb_hTb[:, KH, :], 0.0)
        nc.vector.memset(sb_hTb[:1, KH, :], 1.0)
        nc.vector.tensor_copy(sb_hTb[:, :KH, :], sb_hT[:])
        nc.sync.dma_start(d_adiag.ap(), d_adz.ap())
        nc.sync.dma_start(d_ybounce.ap().rearrange("a b -> (a b)"),
                          d_adz.ap().rearrange("a b c -> (a b c)"))
        nc.sync.dma_start(d_gbounce.ap().rearrange("a b -> (a b)"),
                          d_adz.ap().rearrange("a b c -> (a b c)"))

        with tc.tile_pool(name="imgp", bufs=1) as imgp:
            sb_img = imgp.tile([128, BPC * 2, D], dt.bfloat16, tag="img")
            for b in range(BPC):
                for pt in range(2):
                    nc.sync.dma_start(sb_img[:, 2 * b + pt, :],
                                      d_img[b, 128 * pt:128 * (pt + 1), :])

            # ---------------- phase A: att1w + emb_pre (to DRAM) ----------------
            with tc.tile_pool(name="phA", bufs=1) as phA, \
                 tc.tile_pool(name="phA2", bufs=2) as phA2, \
                 tc.tile_pool(name="phAw", bufs=1) as phAw, \
                 tc.tile_pool(name="phAps", bufs=2, space="PSUM") as phAps:
                sb_encw = phAw.tile([128, KD, A], dt.bfloat16, tag="bigw")
                nc.sync.dma_start(sb_encw[:], d_encw.ap().rearrange("k p a -> p k a"))
                sb_a1b = phAw.tile([128, KA], dt.float32, tag="a1b")
                nc.sync.dma_start(sb_a1b[:], d_att1b.ap().rearrange("(k p) -> p k", p=128))

                for b in range(BPC):
                    imgT = phA.tile([128, KD, PP], dt.bfloat16, tag="imgT")
                    nc.sync.dma_start_transpose(imgT[:], d_img[b, :, :])
                    for at in range(KA):
                        ps = phAps.tile([128, P], dt.float32, tag="a1ps")
                        for kt in range(KD):
                            nc.tensor.matmul(ps[:], sb_encw[:, kt, 128 * at:128 * (at + 1)],
                                             imgT[:, kt, :P], start=(kt == 0), stop=(kt == KD - 1))
                        ev = phA2.tile([128, P], dt.bfloat16, tag="a1ev")
                        nc.scalar.activation(ev[:], ps[:], AF.Identity,
                                             bias=sb_a1b[:, at:at + 1])
                        nc.sync.dma_start(d_att1w[b, :, at, :], ev[:])

                sb_bg = phAw.tile([128, KD], dt.float32, tag="bg")
                nc.sync.dma_start(sb_bg[:], d_bg.ap().rearrange("(m p) -> p m", p=128))
                sb_wihe = phAw.tile([128, KH, NG], dt.bfloat16, tag="bigw")
                nc.sync.dma_start(sb_wihe[:], d_wihe.ap().rearrange("k p g -> p k g"))
                sb_embT = phA.tile([128, KH, T * BPC], dt.bfloat16, tag="imgT")
                nc.sync.dma_start(sb_embT[:], d_embT.ap().rearrange("(k p) x -> p k x", p=128, k=KH))
                TCH = [(0, 8), (8, 8), (16, 8), (24, 7)]
                for gt in range(KD):
                    for (tt0, tn) in TCH:
                        ps = phAps.tile([128, tn * BPC], dt.float32, tag="embps")
                        cols = slice(tt0 * BPC, (tt0 + tn) * BPC)
                        for kt in range(KH):
                            nc.tensor.matmul(ps[:], sb_wihe[:, kt, 128 * gt:128 * (gt + 1)],
                                             sb_embT[:, kt, cols], start=(kt == 0), stop=(kt == KH - 1))
                        ev = phA2.tile([128, tn * BPC], dt.bfloat16, tag="embev")
                        nc.scalar.activation(ev[:], ps[:], AF.Identity, bias=sb_bg[:, gt:gt + 1])
                        nc.sync.dma_start(d_embpre[tt0:tt0 + tn, :, gt, :].rearrange("t p b -> p t b"), ev[:])

            # ---------------- scan ----------------
            ctx2 = ExitStack()
            sc = ctx2.enter_context(tc.tile_pool(name="sc", bufs=2))
            row = ctx2.enter_context(tc.tile_pool(name="row", bufs=2))
            a1p = ctx2.enter_context(tc.tile_pool(name="a1p", bufs=2))
            scw = ctx2.enter_context(tc.tile_pool(name="scw", bufs=3))
            ps_big = ctx2.enter_context(tc.tile_pool(name="psb", bufs=3, space="PSUM"))
            ps_sm = ctx2.enter_context(tc.tile_pool(name="pss", bufs=2, space="PSUM"))


            for t in range(steps):
                n = ns[t]
                embp = sc.tile([128, KD, BPC], dt.bfloat16, tag="embp")
                nc.sync.dma_start(embp[:], d_embpre[t].rearrange("p k b -> p k b"))
                w2ps = []
                for kt in range(KD):
                    w2p = scw.tile([128, NG], dt.bfloat16, tag="w2", name=f"w2_{t}_{kt}")
                    nc.sync.dma_start(w2p[:], d_W2[kt])
                    w2ps.append(w2p)

                # att2wT = dec_att_wf.T @ h   [128,(KA,n)]
                ps_a2 = ps_sm.tile([128, KA, BPC], dt.float32, tag="sm")
                for at in range(KA):
                    for kt in range(KH):
                        nc.tensor.matmul(ps_a2[:, at, :n], sb_decw[:, kt, 128 * at:128 * (at + 1)],
                                         sb_hTb[:, kt, :n], start=(kt == 0), stop=(kt == KH - 1))
                a2 = sc.tile([128, KA, BPC], dt.float32, tag="a2")
                nc.scalar.copy(a2[:, :, :n], ps_a2[:, :, :n])

                # gate preact (f_beta_aug ones-row bias) early: only needs h
                ps_g = [ps_big.tile([BPC, D // 2], dt.float32, tag="big", name=f"ps_g{t}_{i}") for i in range(2)]
                for hf in range(2):
                    for chunk in range(2):
                        cl = slice(hf * 1024 + chunk * 512, hf * 1024 + (chunk + 1) * 512)
                        ocl = slice(chunk * 512, (chunk + 1) * 512)
                        for kt in range(KH + 1):
                            kp = 128 if kt < KH else 1
                            nc.tensor.matmul(ps_g[hf][:n, ocl], sb_hTb[:kp, kt, :n],
                                             sb_fbe[:kp, kt, cl], start=(kt == 0), stop=(kt == KH))
                th_row = row.tile([32, NG], dt.bfloat16, tag="rowb")
                for hf in range(2):
                    hcl = slice(hf * 1024, (hf + 1) * 1024)
                    nc.scalar.activation(th_row[:n, hcl], ps_g[hf][:n], AF.Tanh, scale=0.5)
                g_row = row.tile([32, NG], dt.bfloat16, tag="rowb")
                nc.vector.tensor_scalar(g_row[:n], th_row[:n], 0.5, 0.5, op0=OP.mult, op1=OP.add)

                # score rows (batch-major in PSUM via signs_diag)
                ps_sc = ps_sm.tile([BPC, P], dt.float32, tag="sm")
                mm = 0
                for b in range(n):
                    a1 = a1p.tile([128, KA, P], dt.bfloat16, tag="a1")
                    nc.sync.dma_start(a1[:], d_att1w[b])
                    ys = a1p.tile([128, KA, P], dt.bfloat16, tag="ys")
                    for at in range(KA):
                        nc.vector.tensor_scalar(ys[:, at, :], a1[:, at, :],
                                                a2[:, at, b:b + 1], 0.0,
                                                op0=OP.add, op1=OP.max)
                    for at in range(KA):
                        nc.tensor.matmul(ps_sc[:], sb_sdiag[:, at, b, :], ys[:, at, :],
                                         start=(mm == 0), stop=(mm == 4 * n - 1))
                        mm += 1

                # softmax over p
                nmx = sc.tile([BPC, 1], dt.float32, tag="nmx")
                nc.vector.tensor_reduce(nmx[:n], ps_sc[:n], axis=AX.X, op=OP.max, negate=True)
                ex = sc.tile([BPC, PP], dt.float32, tag="ex")
                nc.scalar.activation(ex[:n, :P], ps_sc[:n], AF.Exp, bias=nmx[:n])
                nc.vector.memset(ex[:n, P:], 0.0)
                sm = sc.tile([BPC, 1], dt.float32, tag="smx")
                nc.vector.tensor_reduce(sm[:n], ex[:n, :P], axis=AX.X, op=OP.add)
                rz = sc.tile([BPC, 1], dt.float32, tag="rz")
                nc.vector.reciprocal(rz[:n], sm[:n])
                al = sc.tile([BPC, P], dt.float32, tag="al")
                nc.vector.tensor_scalar(al[:n], ex[:n, :P], rz[:n], None, op0=OP.mult)
                nc.sync.dma_start(d_alphas[t, :n, :], al[:n])
                exb = sc.tile([BPC, PP], dt.bfloat16, tag="exb")
                nc.vector.tensor_scalar(exb[:n], ex[:n], rz[:n], None, op0=OP.mult)
                # block-diagonal scatter: adiag[2b+h, b, :] = exb[b, 128h:128(h+1)]
                diag_out = bass.AP(d_adiag, 0, [[2 * BPC * 128 + 128, n], [BPC * 128, 2], [1, 128]])
                nc.sync.dma_start(diag_out, exb[:n].rearrange("b (h p) -> b h p", p=128))
                nc.sync.dma_start_transpose(
                    sb_Ablk[:, :2 * n, :].rearrange("p a b -> p (a b)"),
                    d_adiag.ap().rearrange("k c p -> (k c) p")[:2 * n * BPC, :])

                # ctx (unnormalized) = Ablk.T @ img
                ps_ctx = [ps_big.tile([BPC, D // 2], dt.float32, tag="big", name=f"ps_ctx{t}_{i}") for i in range(2)]
                for hf in range(2):
                    for chunk in range(2):
                        cl = slice(hf * 1024 + chunk * 512, hf * 1024 + (chunk + 1) * 512)
                        ocl = slice(chunk * 512, (chunk + 1) * 512)
                        for kt in range(2 * n):
                            nc.tensor.matmul(ps_ctx[hf][:n, ocl], sb_Ablk[:, kt, :n],
                                             sb_img[:, kt, cl],
                                             start=(kt == 0), stop=(kt == 2 * n - 1))

                y_row = row.tile([32, NG], dt.bfloat16, tag="rowb")
                for hf in range(2):
                    hcl = slice(hf * 1024, (hf + 1) * 1024)
                    nc.vector.tensor_mul(y_row[:n, hcl], ps_ctx[hf][:n], g_row[:n, hcl])
                nc.sync.dma_start(d_ybounce.ap()[:n], y_row[:n])
                yT = sc.tile([128, KD, 32], dt.bfloat16, tag="yT")
                nc.sync.dma_start_transpose(yT[:], d_ybounce.ap())

                # gates = yT.T @ W2 (streamed) + hT.T @ w_hh_aug
                ps_gt = [ps_big.tile([BPC, NG // 2], dt.float32, tag="big", name=f"ps_gt{t}_{i}") for i in range(2)]
                for kt in range(KD):
                    for hf in range(2):
                        for chunk in range(2):
                            ocl = slice(chunk * 512, (chunk + 1) * 512)
                            gcl = slice(hf * 1024 + chunk * 512, hf * 1024 + (chunk + 1) * 512)
                            nc.tensor.matmul(ps_gt[hf][:n, ocl], yT[:, kt, :n],
                                             w2ps[kt][:, gcl],
                                             start=(kt == 0), stop=False)
                for kt in range(KH):
                    whp = scw.tile([128, NG], dt.bfloat16, tag="w2", name=f"wh_{t}_{kt}")
                    nc.sync.dma_start(whp[:], d_whh[kt])
                    for hf in range(2):
                        for chunk in range(2):
                            ocl = slice(chunk * 512, (chunk + 1) * 512)
                            gcl = slice(hf * 1024 + chunk * 512, hf * 1024 + (chunk + 1) * 512)
                            nc.tensor.matmul(ps_gt[hf][:n, ocl], sb_hTb[:128, kt, :n],
                                             whp[:, gcl], start=False, stop=(kt == KH - 1))
                gr = row.tile([32, NG], dt.bfloat16, tag="rowb")
                for hf in range(2):
                    nc.vector.tensor_copy(gr[:n, hf * 1024:(hf + 1) * 1024], ps_gt[hf][:n])
                nc.sync.dma_start(d_gbounce.ap()[:n], gr[:n])
                gT = sc.tile([128, KD, 32], dt.bfloat16, tag="gT")
                nc.sync.dma_start_transpose(gT[:], d_gbounce.ap())
                gTe = sc.tile([128, KD, BPC], dt.float32, tag="gTe")
                nc.vector.tensor_add(gTe[:, :, :n], gT[:, :, :n], embp[:, :, :n])

                # LSTM pointwise (d-major)
                sig_i = sc.tile([128, KH, BPC], dt.float32, tag="sig_i")
                nc.scalar.activation(sig_i[:, :, :n], gTe[:, 0:KH, :n], AF.Tanh, scale=0.5)
                nc.vector.tensor_scalar(sig_i[:, :, :n], sig_i[:, :, :n], 0.5, 0.5, op0=OP.mult, op1=OP.add)
                sig_f = sc.tile([128, KH, BPC], dt.float32, tag="sig_f")
                nc.scalar.activation(sig_f[:, :, :n], gTe[:, KH:2 * KH, :n], AF.Tanh, scale=0.5)
                nc.vector.tensor_scalar(sig_f[:, :, :n], sig_f[:, :, :n], 0.5, 0.5, op0=OP.mult, op1=OP.add)
                tan_g = sc.tile([128, KH, BPC], dt.float32, tag="tan_g")
                nc.scalar.activation(tan_g[:, :, :n], gTe[:, 2 * KH:3 * KH, :n], AF.Tanh)
                sig_o = sc.tile([128, KH, BPC], dt.float32, tag="sig_o")
                nc.scalar.activation(sig_o[:, :, :n], gTe[:, 3 * KH:4 * KH, :n], AF.Tanh, scale=0.5)
                nc.vector.tensor_scalar(sig_o[:, :, :n], sig_o[:, :, :n], 0.5, 0.5, op0=OP.mult, op1=OP.add)
                t1 = sc.tile([128, KH, BPC], dt.float32, tag="t1")
                nc.vector.tensor_mul(t1[:, :, :n], sig_f[:, :, :n], sb_cT[:, :, :n])
                t2 = sc.tile([128, KH, BPC], dt.float32, tag="t2")
                nc.vector.tensor_mul(t2[:, :, :n], sig_i[:, :, :n], tan_g[:, :, :n])
                nc.vector.tensor_add(sb_cT[:, :, :n], t1[:, :, :n], t2[:, :, :n])
                tc_ = sc.tile([128, KH, BPC], dt.float32, tag="tc_")
                nc.scalar.activation(tc_[:, :, :n], sb_cT[:, :, :n], AF.Tanh)
                nc.vector.tensor_mul(sb_hT[:, :, :n], sig_o[:, :, :n], tc_[:, :, :n])
                nc.vector.tensor_copy(sb_hTb[:, :KH, :n], sb_hT[:, :, :n])
                nc.vector.tensor_copy(sb_hist[:, :, int(roff[t]):int(roff[t]) + n], sb_hTb[:, :KH, :n])
            ctx2.close()

        # ---------------- FC ----------------
        with tc.tile_pool(name="fcp", bufs=3) as fcp, \
             tc.tile_pool(name="fcps", bufs=4, space="PSUM") as fcps:
            sb_fcb = fcp.tile([128, VT], dt.float32, tag="fcb")
            nc.sync.dma_start(sb_fcb[:], d_fcb.ap().rearrange("(m p) -> p m", p=128))
            for mt in range(VT):
                fw = fcp.tile([128, KH, 128], dt.bfloat16, tag="fw")
                nc.sync.dma_start(fw[:], d_fcw[mt].rearrange("k p v -> p k v"))
                ps = fcps.tile([128, R], dt.float32, tag="fps")
                for kt in range(KH):
                    nc.tensor.matmul(ps[:], fw[:, kt, :], sb_hist[:, kt, :],
                                     start=(kt == 0), stop=(kt == KH - 1))
                ev = fcp.tile([128, R], dt.float32, tag="fev")
                if mt % 2 == 0:
                    nc.scalar.activation(ev[:], ps[:], AF.Identity, bias=sb_fcb[:, mt:mt + 1])
                else:
                    nc.vector.tensor_scalar(ev[:], ps[:], sb_fcb[:, mt:mt + 1], None, op0=OP.add)
                nc.sync.dma_start(d_predsT[128 * mt:128 * (mt + 1), :], ev[:])

    nc.compile()
    return nc


def kernel(**inputs):
    from concourse.bass_utils import run_bass_kernel_spmd

    shared, per_core, meta = _host_prep(inputs)
    ns, roff, R = meta["ns"], meta["roff"], meta["R"]

    key = (tuple(ns), R)
    if key not in _CACHE:
        _CACHE[key] = _build_program(ns, roff, R)
    nc = _CACHE[key]

    in_maps = []
    for c in range(NCORES):
        m = dict(shared)
        m.update(per_core[c])
        in_maps.append(m)

    res = run_bass_kernel_spmd(nc, in_maps, core_ids=list(range(NCORES)))
    kernel.last_results = res

    order, caps, dec_len = meta["order"], meta["caps"], meta["dec_len"]
    predictions = np.zeros((B, T, V), np.float32)
    alphas = np.zeros((B, T, P), np.float32)
    for c in range(NCORES):
        predsT = np.asarray(res.results[c]["predsT"])
        al = np.asarray(res.results[c]["alphas"])
        dl = dec_len[c::NCORES]
        for t in range(T):
            nct = min(int((dl > t).sum()), ns[t])
            if nct == 0:
                continue
            gb = c + NCORES * np.arange(nct)
            predictions[gb, t, :] = predsT[:V, int(roff[t]):int(roff[t]) + nct].T
            alphas[gb, t, :] = al[t, :nct, :]

    return predictions, caps, dec_len, alphas, order
